# revision 5
# baseline (speedup 1.0000x reference)
"""Bidirectional char-LSTM final-hidden kernel for Trainium2 (8 NeuronCores).

Strategy
--------
Data-parallel over the word axis with length-sorted packing:

* Words are bucketed by length; each bucket is padded (by duplicating a
  word) to a multiple of 8 and dealt round-robin to the 8 cores, so every
  core sees an IDENTICAL length composition -> one SPMD program whose
  loop bounds / capture schedule are compile-time constants.
* Per core, words are processed in 512-wide tiles sorted by length; a
  tile runs max(len in tile) LSTM steps.  The backward direction is a
  forward scan over host-reversed sequences, so fwd/bwd share one code
  path and are interleaved to hide per-step latency.
* On-chip layout is transposed: gates/h/c are [128 units, words].  The
  LSTM weights are the stationary matmul operand, activations the moving
  operand, so no on-chip transposes are needed (the host pre-transposes
  x slabs to [char, word]).
* Each word's final hidden state is DMA-captured from the f32 h tile at
  step len-1 (matmul columns are independent, so columns past their
  length just compute garbage that is never read).
"""

from contextlib import ExitStack

import numpy as np
import ml_dtypes

import concourse.bass as bass
import concourse.mybir as mybir
import concourse.tile as tile
from concourse import bacc
from concourse.bass_utils import run_bass_kernel_spmd

N_CORES = 8
TILE_W = 512  # words per tile (moving-operand free dim)
BF16 = mybir.dt.bfloat16
F32 = mybir.dt.float32
_NPBF16 = ml_dtypes.bfloat16


# --------------------------------------------------------------------------
# host-side planning
# --------------------------------------------------------------------------
def _plan(lengths: np.ndarray):
    """Build per-core column->word maps and the (shared) step/capture plan.

    Returns dict with:
      col_word  [N_CORES, T*TILE_W] int32, -1 for dummy columns
      col_len   [T*TILE_W] per-core column lengths (identical across cores)
      n_steps   [T] steps per tile
      offs      [T] step-slab offsets per tile
      captures  list per tile of (step, a, b) column ranges to capture
    """
    W = lengths.shape[0]
    assert W % N_CORES == 0
    maxlen = int(lengths.max())

    dealt = [[] for _ in range(N_CORES)]
    for v in range(1, maxlen + 1):
        idx = np.nonzero(lengths == v)[0]
        if idx.size == 0:
            continue
        pad = (-idx.size) % N_CORES
        if pad:
            idx = np.concatenate([idx, np.repeat(idx[:1], pad)])
        for c in range(N_CORES):
            dealt[c].append(idx[c::N_CORES])
    per_core = [np.concatenate(d) for d in dealt]
    n0 = per_core[0].size
    assert all(p.size == n0 for p in per_core)

    dummy = (-n0) % TILE_W
    ncols = n0 + dummy
    T = ncols // TILE_W

    col_word = np.full((N_CORES, ncols), -1, dtype=np.int64)
    for c in range(N_CORES):
        col_word[c, dummy:] = per_core[c]

    col_len = np.ones(ncols, dtype=np.int64)
    col_len[dummy:] = lengths[per_core[0]]
    for c in range(1, N_CORES):
        assert np.array_equal(col_len[dummy:], lengths[per_core[c]]), (
            "per-core length composition mismatch"
        )

    n_steps, captures = [], []
    for t in range(T):
        seg = col_len[t * TILE_W : (t + 1) * TILE_W]
        n_steps.append(int(seg.max()))
        caps = []
        for v in np.unique(seg):
            pos = np.nonzero(seg == v)[0]
            a, b = int(pos[0]), int(pos[-1]) + 1
            assert b - a == pos.size, "sorted segment not contiguous"
            caps.append((int(v) - 1, a, b))
        captures.append(caps)

    offs = np.concatenate([[0], np.cumsum(n_steps)[:-1]]).astype(int)
    return dict(
        col_word=col_word,
        col_len=col_len,
        n_steps=n_steps,
        offs=offs,
        captures=captures,
        T=T,
        S=int(np.sum(n_steps)),
    )


def _pack_x(x: np.ndarray, plan, core: int):
    """Build the fwd/bwd step slabs [S, n_chars, TILE_W] bf16 for one core."""
    ncols = plan["col_word"].shape[1]
    T, S = plan["T"], plan["S"]
    n_chars = x.shape[2]
    maxlen = x.shape[1]

    ids = plan["col_word"][core]
    safe = np.maximum(ids, 0)
    xc = x[safe]  # [ncols, maxlen, n_chars]
    L = plan["col_len"]

    # reversed-sequence view: xr[j, s] = xc[j, L[j]-1-s]  (s >= L[j]: junk)
    sidx = np.clip(L[:, None] - 1 - np.arange(maxlen)[None, :], 0, maxlen - 1)
    xr = xc[np.arange(ncols)[:, None], sidx]

    xf = np.empty((S, n_chars, TILE_W), dtype=_NPBF16)
    xb = np.empty((S, n_chars, TILE_W), dtype=_NPBF16)
    for t in range(T):
        ns, off = plan["n_steps"][t], plan["offs"][t]
        blk = slice(t * TILE_W, (t + 1) * TILE_W)
        xf[off : off + ns] = xc[blk, :ns, :].transpose(1, 2, 0).astype(_NPBF16)
        xb[off : off + ns] = xr[blk, :ns, :].transpose(1, 2, 0).astype(_NPBF16)
    return xf, xb


# --------------------------------------------------------------------------
# device program
# --------------------------------------------------------------------------
def _build_program(plan, n_chars: int, hidden: int):
    """Emit the SPMD Bass program (identical across cores)."""
    assert n_chars == 128 and hidden == 256, "kernel specialized to 128/256"
    T, S = plan["T"], plan["S"]
    n_steps, offs, captures = plan["n_steps"], plan["offs"], plan["captures"]
    H2 = 2 * TILE_W  # h/c free size: 2 k-chunks of TILE_W words
    SIG = mybir.ActivationFunctionType.Sigmoid
    TANH = mybir.ActivationFunctionType.Tanh

    nc = bacc.Bacc(None, target_bir_lowering=False)
    xd, wih_d, whh_d, bias_d = {}, {}, {}, {}
    for d, nm in ((0, "f"), (1, "b")):
        xd[d] = nc.dram_tensor(f"x_{nm}", [S, 128, TILE_W], BF16, kind="ExternalInput")
        wih_d[d] = nc.dram_tensor(f"wih_{nm}", [128, 1024], BF16, kind="ExternalInput")
        whh_d[d] = nc.dram_tensor(
            f"whh_{nm}", [128, 2, 1024], BF16, kind="ExternalInput"
        )
        bias_d[d] = nc.dram_tensor(f"bias_{nm}", [128, 8], F32, kind="ExternalInput")
    outT = nc.dram_tensor("outT", [512, T * TILE_W], F32, kind="ExternalOutput")
    # [dir, partition, chunk, word-col] view of the output
    outv = outT.rearrange("(d k p) w -> d p k w", d=2, k=2, p=128)

    with tile.TileContext(nc) as tc, ExitStack() as stack:
        wp = stack.enter_context(tc.tile_pool(name="weights", bufs=1))
        xp = stack.enter_context(tc.tile_pool(name="x", bufs=4))
        pp = stack.enter_context(tc.tile_pool(name="psum", bufs=1, space="PSUM"))
        ap = stack.enter_context(tc.tile_pool(name="acts", bufs=2))
        sp = stack.enter_context(tc.tile_pool(name="state", bufs=2))

        wih, whh, bia = {}, {}, {}
        for d in (0, 1):
            wih[d] = wp.tile([128, 1024], BF16, tag=f"wih{d}", name=f"wih{d}")
            nc.sync.dma_start(out=wih[d][:], in_=wih_d[d][:])
            whh[d] = wp.tile([128, 2, 1024], BF16, tag=f"whh{d}", name=f"whh{d}")
            nc.sync.dma_start(out=whh[d][:], in_=whh_d[d][:])
            bia[d] = wp.tile([128, 8], F32, tag=f"bias{d}", name=f"bias{d}")
            nc.sync.dma_start(out=bia[d][:], in_=bias_d[d][:])

        h_prev = {0: None, 1: None}
        c_prev = {0: None, 1: None}

        for t in range(T):
            ns = n_steps[t]
            caps_by_step = {}
            for s_, a_, b_ in captures[t]:
                caps_by_step.setdefault(s_, []).append((a_, b_))
            for s in range(ns):
                for d in (0, 1):
                    x_sb = xp.tile([128, TILE_W], BF16, tag=f"x{d}", bufs=4)
                    nc.sync.dma_start(out=x_sb[:], in_=xd[d][offs[t] + s])

                    ps = []
                    for q in range(8):
                        pq = pp.tile([128, TILE_W], F32, tag=f"ps{q}")
                        ps.append(pq)
                        lhs_x = wih[d][:, q * 128 : (q + 1) * 128]
                        nc.tensor.matmul(
                            pq[:], lhs_x, x_sb[:], start=True, stop=(s == 0)
                        )
                        if s > 0:
                            hT = h_prev[d]
                            for k in (0, 1):
                                nc.tensor.matmul(
                                    pq[:],
                                    whh[d][:, k, q * 128 : (q + 1) * 128],
                                    hT[:, k * TILE_W : (k + 1) * TILE_W],
                                    start=False,
                                    stop=(k == 1),
                                )

                    sig_i = ap.tile([128, H2], BF16, tag=f"si{d}")
                    sig_f = ap.tile([128, H2], F32, tag=f"sf{d}")
                    tan_g = ap.tile([128, H2], BF16, tag=f"tg{d}")
                    sig_o = ap.tile([128, H2], BF16, tag=f"so{d}")
                    gates = (sig_i, sig_i, sig_f, sig_f, tan_g, tan_g, sig_o, sig_o)
                    funcs = (SIG, SIG, SIG, SIG, TANH, TANH, SIG, SIG)
                    for q in range(8):
                        half = q % 2
                        nc.scalar.activation(
                            out=gates[q][:, half * TILE_W : (half + 1) * TILE_W],
                            in_=ps[q][:],
                            func=funcs[q],
                            bias=bia[d][:, q : q + 1],
                        )

                    c_new = sp.tile([128, H2], F32, tag=f"c{d}")
                    if s == 0:
                        nc.vector.tensor_mul(c_new[:], sig_i[:], tan_g[:])
                    else:
                        t2 = sp.tile([128, H2], BF16, tag=f"t2{d}")
                        nc.vector.tensor_mul(t2[:], sig_i[:], tan_g[:])
                        t1 = sp.tile([128, H2], F32, tag=f"t1{d}")
                        nc.vector.tensor_mul(t1[:], sig_f[:], c_prev[d][:])
                        nc.vector.tensor_add(c_new[:], t1[:], t2[:])
                    c_prev[d] = c_new

                    tan_c = sp.tile([128, H2], F32, tag=f"tc{d}")
                    nc.scalar.activation(out=tan_c[:], in_=c_new[:], func=TANH)
                    h_f32 = sp.tile([128, H2], F32, tag=f"hf{d}")
                    nc.vector.tensor_mul(h_f32[:], sig_o[:], tan_c[:])

                    if s < ns - 1:
                        h_bf = sp.tile([128, H2], BF16, tag=f"h{d}")
                        nc.gpsimd.tensor_copy(out=h_bf[:], in_=h_f32[:])
                        h_prev[d] = h_bf

                    for a, b in caps_by_step.get(s, ()):
                        hv = h_f32.rearrange("p (k w) -> p k w", k=2)
                        nc.sync.dma_start(
                            out=outv[d, :, :, t * TILE_W + a : t * TILE_W + b],
                            in_=hv[:, :, a:b],
                        )

    nc.compile()
    return nc


# --------------------------------------------------------------------------
# entry point
# --------------------------------------------------------------------------
def _prep_weights(Wih, Whh, bih, bhh):
    wihT = np.ascontiguousarray(Wih.T).astype(_NPBF16)  # [128, 1024]
    whhT = np.ascontiguousarray(
        Whh.T.reshape(2, 128, 1024).transpose(1, 0, 2)
    ).astype(_NPBF16)  # [128, 2, 1024]
    bias = (bih + bhh).astype(np.float32).reshape(8, 128).T  # [128, 8]
    return wihT, whhT, np.ascontiguousarray(bias)


def kernel(x, lengths, Wih_f, Whh_f, bih_f, bhh_f, Wih_b, Whh_b, bih_b, bhh_b):
    x = np.asarray(x, dtype=np.float32)
    lengths = np.asarray(lengths)
    n_words, maxlen, n_chars = x.shape
    hidden = Whh_f.shape[1]

    plan = _plan(lengths.astype(np.int64))
    nc = _build_program(plan, n_chars, hidden)

    wf = _prep_weights(Wih_f, Whh_f, bih_f, bhh_f)
    wb = _prep_weights(Wih_b, Whh_b, bih_b, bhh_b)

    in_maps = []
    for c in range(N_CORES):
        xf, xb = _pack_x(x, plan, c)
        in_maps.append(
            {
                "x_f": xf,
                "x_b": xb,
                "wih_f": wf[0],
                "whh_f": wf[1],
                "bias_f": wf[2],
                "wih_b": wb[0],
                "whh_b": wb[1],
                "bias_b": wb[2],
            }
        )

    res = run_bass_kernel_spmd(nc, in_maps, core_ids=list(range(N_CORES)))

    out = np.empty((n_words, 2 * hidden), dtype=np.float32)
    for c in range(N_CORES):
        ids = plan["col_word"][c]
        keep = ids >= 0
        out[ids[keep]] = res.results[c]["outT"][:, keep].T
    # stash internals for test harnesses
    kernel._last = dict(nc=nc, plan=plan, in_maps=in_maps)
    return out
